# revision 29
# baseline (speedup 1.0000x reference)
"""Multi-head attention (B=4, S=2048, D=1024, 16 heads x 64) on 8 NeuronCores.

Sharding: DP=4 over batch x TP=2 over heads (8 heads/core).
Each core computes, for one batch element and half the heads:
    qhT = (q @ Wq + bq)^T       [512, 2048]   (bf16, head-dim on partitions)
    khT = (k @ Wk + bk)^T       [512, 2048]
    vh  = v @ Wv                [2048, 512]   (natural layout, k on partitions)
    per (head-pair, q-chunk): flash-style S^T = kh @ qh^T, p = exp(scale*S^T),
      outT_u = vh^T @ p (col-tiled pair) and l = ones^T @ p (replicated rows),
      outT = outT_u * approx_recip(l)
    partial_out = outT^T @ Wo_shard          [2048, 1024]  (fp32)
Host sums the TP pair partials and adds the bias terms (bv @ Wo + bo).

Inputs are transposed + cast to bf16 on the host so no on-device transposes
are needed (matmul contraction must be on the partition dim for both operands).
"""

import os
import sys

sys.path.insert(0, "/opt/trn_rl_repo")

import numpy as np
import ml_dtypes

S = 2048          # sequence length
DM = 1024         # model dim
HD = 512          # local head-dim total (8 heads x 64) per core (TP=2)
NB = 4            # batch
NCORES = 8
P = 128
DK = 64
SCALE = 1.0 / 8.0  # 1/sqrt(64)

_CACHE = {}


def _build_nc():
    import concourse.bass as bass  # noqa: F401
    import concourse.mybir as mybir
    from concourse import bacc, tile
    from contextlib import ExitStack

    BF = mybir.dt.bfloat16
    F32 = mybir.dt.float32
    Exp = mybir.ActivationFunctionType.Exp

    nc = bacc.Bacc("TRN2", target_bir_lowering=False, debug=False, num_swdge_queues=4)

    qT = nc.dram_tensor("qT", [DM, S], BF, kind="ExternalInput")
    kT = nc.dram_tensor("kT", [DM, S], BF, kind="ExternalInput")
    vT = nc.dram_tensor("vT", [DM, S], BF, kind="ExternalInput")
    wq = nc.dram_tensor("wq", [DM, HD], BF, kind="ExternalInput")
    wk = nc.dram_tensor("wk", [DM, HD], BF, kind="ExternalInput")
    wv = nc.dram_tensor("wv", [DM, HD], BF, kind="ExternalInput")
    wo = nc.dram_tensor("wo", [HD, DM], BF, kind="ExternalInput")
    bq = nc.dram_tensor("bq", [HD], F32, kind="ExternalInput")
    bk = nc.dram_tensor("bk", [HD], F32, kind="ExternalInput")
    out = nc.dram_tensor("out", [S, DM], F32, kind="ExternalOutput")

    NM = DM // P      # 8 m-chunks
    NHP = HD // P     # 4 head pairs
    NSC = S // 512    # 4 s-chunks of 512
    NJ = S // P       # 16 k-chunks

    with ExitStack() as ctx:
        tc = ctx.enter_context(tile.TileContext(nc))

        const = ctx.enter_context(tc.tile_pool(name="const", bufs=1))
        wq_pool = ctx.enter_context(tc.tile_pool(name="wq_pool", bufs=8))
        wk_pool = ctx.enter_context(tc.tile_pool(name="wk_pool", bufs=8))
        wv_pool = ctx.enter_context(tc.tile_pool(name="wv_pool", bufs=8))
        wo_pool = ctx.enter_context(tc.tile_pool(name="wo_pool", bufs=4))
        inpool = ctx.enter_context(tc.tile_pool(name="inpool", bufs=16))
        qh_pool = ctx.enter_context(tc.tile_pool(name="qh_pool", bufs=4))
        kh_pool = ctx.enter_context(tc.tile_pool(name="kh_pool", bufs=4))
        vh_pool = ctx.enter_context(tc.tile_pool(name="vh_pool", bufs=16))
        outT_pool = ctx.enter_context(tc.tile_pool(name="outT_pool", bufs=4))
        p_pool = ctx.enter_context(tc.tile_pool(name="p_pool", bufs=6))
        rec_pool = ctx.enter_context(tc.tile_pool(name="rec_pool", bufs=2))
        stage_pool = ctx.enter_context(tc.tile_pool(name="stage_pool", bufs=3))
        st_ps = ctx.enter_context(tc.tile_pool(name="st_ps", bufs=3, space="PSUM"))
        pv_ps = ctx.enter_context(tc.tile_pool(name="pv_ps", bufs=2, space="PSUM"))

        # constants
        ones_t = const.tile([P, DK], BF, tag="ones")
        nc.vector.memset(ones_t[:], 1.0)
        bq_sb = const.tile([P, NHP], F32, tag="bq")
        nc.gpsimd.dma_start(bq_sb[:], bq[:].rearrange("(f p) -> p f", p=P))
        bk_sb = const.tile([P, NHP], F32, tag="bk")
        nc.gpsimd.dma_start(bk_sb[:], bk[:].rearrange("(f p) -> p f", p=P))

        def load_weight(pool, handle, tag, eng=None):
            eng = eng or nc.sync
            tiles = []
            for m in range(NM):
                t = pool.tile([P, HD], BF, tag=tag)
                eng.dma_start(t[:], handle[m * P : (m + 1) * P, :])
                tiles.append(t)
            return tiles

        def load_input(handle, tag, eng=None):
            eng = eng or nc.sync
            tiles = []
            for m in range(NM):
                t = inpool.tile([P, S], BF, tag="in")
                eng.dma_start(t[:], handle[m * P : (m + 1) * P, :])
                tiles.append(t)
            return tiles

        # ---- v projection chains (vh[s, hd] natural layout, 16 k-chunks);
        # first 8 run upfront, the rest ride as fillers in block (0,0) ----
        wv_sb = load_weight(wv_pool, wv, "wv")
        vT_sb = load_input(vT, "vT")
        vh_sb = [vh_pool.tile([P, HD], BF, tag="vh", name=f"vh{i}") for i in range(NJ)]

        def vproj_chain_ops(sc):
            cell = {}

            def mk(m):
                def op():
                    if m == 0:
                        if sc % 3 == 2:
                            cell["ps"] = pv_ps.tile([P, HD], F32, tag="pvps", name="vps2")
                        else:
                            cell["ps"] = st_ps.tile([P, 1024], F32, tag="stps", name="vps")[:, 0:HD]
                    nc.tensor.matmul(
                        cell["ps"][:],
                        lhsT=vT_sb[m][:, sc * P : (sc + 1) * P],
                        rhs=wv_sb[m][:],
                        start=(m == 0),
                        stop=(m == NM - 1),
                    )
                return op

            ops = [mk(m) for m in range(NM)]
            ops.append(lambda: nc.scalar.copy(vh_sb[sc][:], cell["ps"][:]))
            return ops

        # ---- q/k projections: qhT/khT [hd, s], head-pair-major tiles ----
        wk_sb = load_weight(wk_pool, wk, "wk", eng=nc.gpsimd)
        kT_sb = load_input(kT, "kT")
        wq_sb = load_weight(wq_pool, wq, "wq", eng=nc.gpsimd)
        qT_sb = load_input(qT, "qT")

        qhT_sb = [qh_pool.tile([P, S], BF, tag="qh", name=f"qhT{i}") for i in range(NHP)]
        khT_sb = [kh_pool.tile([P, S], BF, tag="kh", name=f"khT{i}") for i in range(NHP)]
        outT_sb = [outT_pool.tile([P, S], BF, tag="outT", name=f"outT{i}") for i in range(NHP)]

        def proj_chain_ops(w_sb, x_sb, dst, bias_sb, hp, sc):
            """One projection output chunk as a list of single-op closures."""
            cell = {}

            def mk(m):
                def op():
                    if m == 0:
                        cell["ps"] = st_ps.tile([P, 1024], F32, tag="stps", name="fps")[:, 0:512]
                    nc.tensor.matmul(
                        cell["ps"][:],
                        lhsT=w_sb[m][:, hp * P : (hp + 1) * P],
                        rhs=x_sb[m][:, sc * 512 : (sc + 1) * 512],
                        start=(m == 0),
                        stop=(m == NM - 1),
                    )
                return op

            ops = [mk(m) for m in range(NM)]

            def ev():
                nc.vector.tensor_scalar_add(
                    dst[:, sc * 512 : (sc + 1) * 512],
                    cell["ps"][:],
                    bias_sb[:, hp : hp + 1],
                )

            ops.append(ev)
            return ops

        def fc_chain_ops(sc, ec):
            ss = slice(sc * P, (sc + 1) * P)
            es = slice(ec * 512, (ec + 1) * 512)
            cell = {}

            def mk(hp):
                def op():
                    if hp == 0:
                        cell["ps"] = st_ps.tile([P, 1024], F32, tag="stps", name="fps")[:, 0:512]
                    nc.tensor.matmul(
                        cell["ps"][:],
                        lhsT=outT_sb[hp][:, ss],
                        rhs=wo_sb[hp][:, es],
                        start=(hp == 0),
                        stop=(hp == NHP - 1),
                    )
                return op

            ops = [mk(hp) for hp in range(NHP)]

            def ev():
                stg = stage_pool.tile([P, 512], F32, tag="stg", name="stg")
                nc.vector.tensor_copy(stg[:], cell["ps"][:])
                nc.gpsimd.dma_start(out[ss, es], stg[:])

            ops.append(ev)
            return ops

        from collections import deque

        fillers = deque()

        def drain(n):
            for _ in range(n):
                if not fillers:
                    return
                fillers.popleft()()

        # upfront (inside the input-DMA window): v-proj, kp0 and qp0 chunk 0
        # interleaved round-robin over 5 psum slots so slot-recycle latency
        # amortizes — just enough for block (0,0) to start
        up = [vproj_chain_ops(sc) for sc in range(NJ)]
        for sc in range(NSC):
            up.insert(5 * sc + 4, proj_chain_ops(wk_sb, kT_sb, khT_sb[0], bk_sb, 0, sc))
        up.append(proj_chain_ops(wq_sb, qT_sb, qhT_sb[0], bq_sb, 0, 0))
        for ops in up:
            for op in ops:
                op()
        wo_sb = []
        for hp in range(NHP):
            t = wo_pool.tile([P, DM], BF, tag="wo")
            nc.gpsimd.dma_start(t[:], wo[hp * P : (hp + 1) * P, :])
            wo_sb.append(t)

        # remaining projection work rides along inside the attention blocks
        for sc in range(1, NSC):
            fillers.extend(proj_chain_ops(wq_sb, qT_sb, qhT_sb[0], bq_sb, 0, sc))
        for nhp in range(1, NHP):
            for sc in range(NSC):
                fillers.extend(proj_chain_ops(wk_sb, kT_sb, khT_sb[nhp], bk_sb, nhp, sc))
            for sc in range(NSC):
                fillers.extend(proj_chain_ops(wq_sb, qT_sb, qhT_sb[nhp], bq_sb, nhp, sc))

        # ---- attention, flash style, software-pipelined emission so the PE
        # queue keeps st(j+3) ahead of PV(j): the st absorbs the exp-slot
        # wait and the PV foursome pipelines behind it; block tails spill
        # into the next block's emission so ACT never drains at boundaries. ----
        carry = []  # deferred ops from the previous block

        def attn_block(hp, qc, budget):
            qs = slice(qc * 512, (qc + 1) * 512)
            state = {}
            p_tiles = {}

            def ensure_pv_tiles():
                if "P" not in state:
                    state["P"] = pv_ps.tile([P, 512], F32, tag="pvps", name="Pps")
                    state["L"] = pv_ps.tile([P, 512], F32, tag="pvps", name="Lps")

            def emit_st(j):
                ks = slice(j * P, (j + 1) * P)
                st = st_ps.tile([P, 1024], F32, tag="stps")
                nc.tensor.matmul(
                    st[:, 0:512],
                    lhsT=khT_sb[hp][0:64, ks],
                    rhs=qhT_sb[hp][0:64, qs],
                    start=True,
                    stop=True,
                    tile_position=(0, 0),
                )
                nc.tensor.matmul(
                    st[:, 512:1024],
                    lhsT=khT_sb[hp][64:128, ks],
                    rhs=qhT_sb[hp][64:128, qs],
                    start=True,
                    stop=True,
                    tile_position=(64, 0),
                )
                p = p_pool.tile([P, 1024], BF, tag="p")
                nc.scalar.activation(p[:], st[:], Exp, scale=SCALE)
                p_tiles[j] = p

            def emit_pv(j):
                ensure_pv_tiles()
                P_ps, L_ps = state["P"], state["L"]
                p = p_tiles.pop(j)
                first, last = (j == 0), (j == NJ - 1)
                nc.tensor.matmul(
                    P_ps[0:64, :],
                    lhsT=vh_sb[j][:, hp * P : hp * P + DK],
                    rhs=p[:, 0:512],
                    start=first,
                    stop=last,
                    tile_position=(0, 0),
                    skip_group_check=True,
                )
                nc.tensor.matmul(
                    P_ps[64:128, :],
                    lhsT=vh_sb[j][:, hp * P + DK : (hp + 1) * P],
                    rhs=p[:, 512:1024],
                    start=first,
                    stop=last,
                    tile_position=(0, 64),
                    skip_group_check=True,
                )
                nc.tensor.matmul(
                    L_ps[0:64, :],
                    lhsT=ones_t[:],
                    rhs=p[:, 0:512],
                    start=first,
                    stop=last,
                    tile_position=(0, 0),
                    skip_group_check=True,
                )
                nc.tensor.matmul(
                    L_ps[64:128, :],
                    lhsT=ones_t[:],
                    rhs=p[:, 512:1024],
                    start=first,
                    stop=last,
                    tile_position=(0, 64),
                    skip_group_check=True,
                )

            LAG = 3
            for j in range(NJ):
                emit_st(j)
                if carry:
                    carry.pop(0)()
                if j >= LAG:
                    emit_pv(j - LAG)
                if not carry:
                    drain(budget)

            def mk_pv(j):
                return lambda: emit_pv(j)

            def normalize():
                rec = rec_pool.tile([P, 512], F32, tag="rec")
                nc.vector.reciprocal_approx_fast(rec[:], state["L"][:])
                nc.vector.tensor_mul(outT_sb[hp][:, qs], state["P"][:], rec[:])

            return [mk_pv(j) for j in range(NJ - LAG, NJ)] + [normalize]

        for hp in range(NHP):
            for qc in range(NSC):
                carry = attn_block(hp, qc, 4 if hp == NHP - 1 else 2)
                if hp == NHP - 1:
                    for sc in range(qc * 4, qc * 4 + 4):
                        fillers.extend(fc_chain_ops(sc, 0))
                        fillers.extend(fc_chain_ops(sc, 1))
        for op in carry:
            op()

        # whatever is left (last fc chunks)
        while fillers:
            fillers.popleft()()

    nc.compile()
    return nc


def _get_nc():
    if "nc" not in _CACHE:
        _CACHE["nc"] = _build_nc()
    return _CACHE["nc"]


def kernel(q, k, v, Wq, bq, Wk, bk, Wv, bv, Wo, bo):
    from concourse.bass_utils import run_bass_kernel_spmd

    bf16 = ml_dtypes.bfloat16
    q, k, v = (np.asarray(x, np.float32) for x in (q, k, v))
    Wq, bq, Wk, bk, Wv, bv, Wo, bo = (
        np.asarray(x, np.float32) for x in (Wq, bq, Wk, bk, Wv, bv, Wo, bo)
    )

    in_maps = []
    for c in range(NCORES):
        b, t = c // 2, c % 2
        hs = slice(t * HD, (t + 1) * HD)
        in_maps.append(
            {
                "qT": q[b].T.astype(bf16),
                "kT": k[b].T.astype(bf16),
                "vT": v[b].T.astype(bf16),
                "wq": Wq[:, hs].astype(bf16),
                "wk": Wk[:, hs].astype(bf16),
                "wv": Wv[:, hs].astype(bf16),
                "wo": Wo[hs, :].astype(bf16),
                "bq": np.ascontiguousarray(bq[hs]),
                "bk": np.ascontiguousarray(bk[hs]),
            }
        )

    nc = _get_nc()
    trace = os.environ.get("KERNEL_TRACE", "0") == "1"
    res = run_bass_kernel_spmd(
        nc, in_maps, core_ids=list(range(NCORES)), trace=trace
    )
    if trace:
        print(f"HW exec time: {res.exec_time_ns} ns")

    host_bias = (bv @ Wo + bo).astype(np.float32)
    full = np.empty((NB, S, DM), np.float32)
    for b in range(NB):
        full[b] = res.results[2 * b]["out"] + res.results[2 * b + 1]["out"] + host_bias
    return full
